# revision 2
# baseline (speedup 1.0000x reference)
"""MoE gate (DeepSeek-style) Trainium2 Bass kernel.

Computation (per token): logits = x @ W.T over 64 experts; softmax; top-8
indices + renormalized top-8 weights; plus the seq-aux-loss statistics
(per-batch expert histogram of the top-8 picks and per-batch mean softmax
scores), reduced to a scalar aux loss.

Sharding: data parallel over tokens. hidden_states [4, 8192, 2048] is
flattened to [32768, 2048] and split into 8 contiguous shards of 4096
tokens (core c gets tokens of batch c//2, so per-batch stats reassemble
trivially on host). The [64, 2048] gate weight is replicated (host passes
it pre-transposed as [2048, 64]).

Per-core kernel (4096 tokens, 32 blocks of 128):
  - DMA x block [128 tok, 2048] to SBUF (1 MiB, contiguous).
  - PE-transposes the block 128x128 at a time into PSUM (fp32 has no DMA
    transpose on TRN2), copies to SBUF via DVE/ACT.
  - 16 accumulating fp32 matmuls: logits[128 tok, 64 exp] in PSUM
    (lhsT = x.T chunk, rhs = W.T chunk).
  - ACT: e = exp(logits) with accumulated row-sum Z (softmax denominator).
  - DVE: max8 -> top-8 values (descending, matches jax.lax.top_k order),
    max_index -> top-8 indices; renormalized weights = m8 / sum(m8)
    (the softmax denominator cancels); running stats via fused
    scalar_tensor_tensor: acc_s += e * (1/Z), acc_h += (e >= m8[:,7]).
  - Outputs staged in SBUF and stored every 8 blocks; the [128, 64] stat
    accumulators are stored once at the end (host sums the 128 partitions).

The host then concatenates the per-core topk outputs and finishes the
tiny aux-loss reduction ([8, 2, 64] -> scalar) in numpy.
"""

import numpy as np

P = 128
DIM = 2048
E = 64
TOPK = 8
BSZ, SEQ = 4, 8192
N_TOKENS = BSZ * SEQ
N_CORES = 8
TOK_PER_CORE = N_TOKENS // N_CORES  # 4096
NBLK = TOK_PER_CORE // P  # 32
NCH = DIM // P  # 16
GRP = 8  # blocks per output store
NGRP = NBLK // GRP
ALPHA = 0.1

_CACHE = {}


def _build_program():
    from contextlib import ExitStack

    import concourse.bacc as bacc
    import concourse.mybir as mybir
    import concourse.tile as tile
    from concourse.masks import make_identity

    f32 = mybir.dt.float32
    u32 = mybir.dt.uint32
    Alu = mybir.AluOpType

    nc = bacc.Bacc("TRN2", target_bir_lowering=False, debug=False,
                   num_devices=N_CORES)
    x = nc.dram_tensor("x", [TOK_PER_CORE, DIM], f32, kind="ExternalInput").ap()
    wt = nc.dram_tensor("wt", [DIM, E], f32, kind="ExternalInput").ap()
    idx_out = nc.dram_tensor("idx_out", [TOK_PER_CORE, TOPK], u32,
                             kind="ExternalOutput").ap()
    wgt_out = nc.dram_tensor("wgt_out", [TOK_PER_CORE, TOPK], f32,
                             kind="ExternalOutput").ap()
    stats_out = nc.dram_tensor("stats_out", [2, P, E], f32,
                               kind="ExternalOutput").ap()

    idx_view = idx_out.rearrange("(g b p) k -> g p b k", g=NGRP, b=GRP, p=P)
    wgt_view = wgt_out.rearrange("(g b p) k -> g p b k", g=NGRP, b=GRP, p=P)

    with tile.TileContext(nc) as tc:
        with ExitStack() as ctx:
            const = ctx.enter_context(tc.tile_pool(name="const", bufs=1))
            xin = ctx.enter_context(tc.tile_pool(name="xin", bufs=4))
            xtp = ctx.enter_context(tc.tile_pool(name="xtp", bufs=3))
            work = ctx.enter_context(tc.tile_pool(name="work", bufs=4))
            small = ctx.enter_context(tc.tile_pool(name="small", bufs=6))
            stage = ctx.enter_context(tc.tile_pool(name="stage", bufs=2))
            psum_t = ctx.enter_context(
                tc.tile_pool(name="psum_t", bufs=6, space="PSUM"))
            psum_l = ctx.enter_context(
                tc.tile_pool(name="psum_l", bufs=2, space="PSUM"))

            identity = const.tile([P, P], f32)
            make_identity(nc, identity)
            wt_s = const.tile([P, NCH, E], f32)
            nc.sync.dma_start(out=wt_s, in_=wt.rearrange("(c p) e -> p c e", p=P))
            acc_s = const.tile([P, E], f32)
            acc_h = const.tile([P, E], f32)
            nc.vector.memset(acc_s, 0.0)
            nc.vector.memset(acc_h, 0.0)

            for g in range(NGRP):
                idx_stage = stage.tile([P, GRP, TOPK], u32, tag="idx_stage")
                wgt_stage = stage.tile([P, GRP, TOPK], f32, tag="wgt_stage")
                for bi in range(GRP):
                    b = g * GRP + bi
                    xt = xin.tile([P, DIM], f32, tag="xin")
                    nc.sync.dma_start(out=xt, in_=x[b * P:(b + 1) * P, :])

                    # Transpose the block chunk-by-chunk: 4 chunks of
                    # [128, 128] per PSUM bank, then one copy per bank.
                    xT = xtp.tile([P, NCH, P], f32, tag="xtp")
                    for q in range(4):
                        pt = psum_t.tile([P, 4, P], f32, tag="pt")
                        for j in range(4):
                            c = q * 4 + j
                            nc.tensor.transpose(
                                pt[:, j, :], xt[:, c * P:(c + 1) * P], identity)
                        dst = xT[:, q * 4:(q + 1) * 4, :]
                        if q % 2 == 0:
                            nc.vector.tensor_copy(out=dst, in_=pt)
                        else:
                            nc.scalar.copy(out=dst, in_=pt)

                    lg = psum_l.tile([P, E], f32, tag="lg")
                    for c in range(NCH):
                        nc.tensor.matmul(lg, xT[:, c, :], wt_s[:, c, :],
                                         start=(c == 0), stop=(c == NCH - 1))

                    e_t = work.tile([P, E], f32, tag="e")
                    z = small.tile([P, 1], f32, tag="z")
                    nc.scalar.activation(e_t, lg,
                                         mybir.ActivationFunctionType.Exp,
                                         accum_out=z)
                    m8 = small.tile([P, TOPK], f32, tag="m8")
                    nc.vector.max(out=m8, in_=e_t)
                    nc.vector.max_index(out=idx_stage[:, bi, :], in_max=m8,
                                        in_values=e_t)
                    s8 = small.tile([P, 1], f32, tag="s8")
                    nc.vector.reduce_sum(s8, m8, axis=mybir.AxisListType.X)
                    r8 = small.tile([P, 1], f32, tag="r8")
                    nc.vector.reciprocal(r8, s8)
                    nc.vector.tensor_scalar_mul(wgt_stage[:, bi, :], m8, r8)
                    rz = small.tile([P, 1], f32, tag="rz")
                    nc.vector.reciprocal(rz, z)
                    # acc_s += e * (1/Z);  acc_h += (e >= 8th-largest e)
                    nc.vector.scalar_tensor_tensor(
                        out=acc_s, in0=e_t, scalar=rz, in1=acc_s,
                        op0=Alu.mult, op1=Alu.add)
                    nc.vector.scalar_tensor_tensor(
                        out=acc_h, in0=e_t, scalar=m8[:, TOPK - 1:TOPK],
                        in1=acc_h, op0=Alu.is_ge, op1=Alu.add)

                nc.sync.dma_start(out=idx_view[g], in_=idx_stage)
                nc.sync.dma_start(out=wgt_view[g], in_=wgt_stage)

            nc.sync.dma_start(out=stats_out[0], in_=acc_s)
            nc.sync.dma_start(out=stats_out[1], in_=acc_h)

    nc.compile()
    return nc


def _get_program():
    if "nc" not in _CACHE:
        _CACHE["nc"] = _build_program()
    return _CACHE["nc"]


def kernel(hidden_states, weight):
    from concourse.bass_utils import run_bass_kernel_spmd

    x = np.ascontiguousarray(np.asarray(hidden_states, dtype=np.float32))
    x = x.reshape(N_TOKENS, DIM)
    w = np.asarray(weight, dtype=np.float32)
    wt = np.ascontiguousarray(w.T)  # [DIM, E]

    nc = _get_program()
    in_maps = [
        {"x": x[c * TOK_PER_CORE:(c + 1) * TOK_PER_CORE], "wt": wt}
        for c in range(N_CORES)
    ]
    res = run_bass_kernel_spmd(nc, in_maps, core_ids=list(range(N_CORES)))
    outs = res.results

    topk_idx = np.concatenate(
        [o["idx_out"].view(np.int32) for o in outs], axis=0)
    topk_wgt = np.concatenate([o["wgt_out"] for o in outs], axis=0)

    # stats: [core, 2, 128, 64] -> per-core column sums -> per-batch sums
    colsums = np.stack([o["stats_out"] for o in outs]).sum(axis=2)  # [8, 2, 64]
    per_batch = colsums.reshape(BSZ, 2, 2, E).sum(axis=1)  # [4, 2, 64]
    s_sum = per_batch[:, 0]  # sum over seq of softmax scores  [4, 64]
    h_sum = per_batch[:, 1]  # top-8 histogram counts          [4, 64]
    ce = h_sum / np.float32(SEQ * TOPK / E)
    smean = s_sum / np.float32(SEQ)
    aux_loss = np.float32((ce * smean).sum(axis=1).mean() * ALPHA)

    return topk_idx, topk_wgt, aux_loss


# revision 6
# speedup vs baseline: 1.0090x; 1.0090x over previous
"""MoE gate (DeepSeek-style) Trainium2 Bass kernel.

Computation (per token): logits = x @ W.T over 64 experts; softmax; top-8
indices + renormalized top-8 weights; plus the seq-aux-loss statistics
(per-batch expert histogram of the top-8 picks and per-batch mean softmax
scores), reduced to a scalar aux loss.

Sharding: data parallel over tokens. hidden_states [4, 8192, 2048] is
flattened to [32768, 2048] and split into 8 contiguous shards of 4096
tokens (core c gets tokens of batch c//2, so per-batch stats reassemble
trivially on host). The [64, 2048] gate weight is replicated (host passes
it pre-transposed as [2048, 64]).

Per-core kernel (4096 tokens, 32 blocks of 128):
  - DMA x block [128 tok, 2048] to SBUF (1 MiB, contiguous).
  - PE-transposes the block 128x128 at a time into PSUM (fp32 has no DMA
    transpose on TRN2), copies to SBUF via DVE/ACT.
  - 16 accumulating fp32 matmuls: logits[128 tok, 64 exp] in PSUM
    (lhsT = x.T chunk, rhs = W.T chunk).
  - ACT: e = exp(logits) with accumulated row-sum Z (softmax denominator).
  - DVE: max8 -> top-8 values (descending, matches jax.lax.top_k order),
    max_index -> top-8 indices; renormalized weights = m8 / sum(m8)
    (the softmax denominator cancels); running stats via fused
    scalar_tensor_tensor: acc_s += e * (1/Z), acc_h += (e >= m8[:,7]).
  - Outputs staged in SBUF and stored every 8 blocks; the [128, 64] stat
    accumulators are stored once at the end (host sums the 128 partitions).

The host then concatenates the per-core topk outputs and finishes the
tiny aux-loss reduction ([8, 2, 64] -> scalar) in numpy.
"""

import numpy as np

P = 128
DIM = 2048
E = 64
TOPK = 8
BSZ, SEQ = 4, 8192
N_TOKENS = BSZ * SEQ
N_CORES = 8
TOK_PER_CORE = N_TOKENS // N_CORES  # 4096
NBLK = TOK_PER_CORE // P  # 32
NCH = DIM // P  # 16
GRP = 8  # blocks per output store
NGRP = NBLK // GRP
ALPHA = 0.1

_CACHE = {}


def _build_program():
    from contextlib import ExitStack

    import concourse.bacc as bacc
    import concourse.mybir as mybir
    import concourse.tile as tile
    from concourse.masks import make_identity

    f32 = mybir.dt.float32
    u32 = mybir.dt.uint32
    Alu = mybir.AluOpType

    nc = bacc.Bacc("TRN2", target_bir_lowering=False, debug=False,
                   num_devices=N_CORES)
    x = nc.dram_tensor("x", [TOK_PER_CORE, DIM], f32, kind="ExternalInput").ap()
    # host passes W.T pre-shuffled to [p, c, e] so this loads as one
    # contiguous DMA with 4 KiB per-partition descriptors
    wt = nc.dram_tensor("wt", [P, NCH, E], f32, kind="ExternalInput").ap()
    idx_out = nc.dram_tensor("idx_out", [TOK_PER_CORE, TOPK], u32,
                             kind="ExternalOutput").ap()
    wgt_out = nc.dram_tensor("wgt_out", [TOK_PER_CORE, TOPK], f32,
                             kind="ExternalOutput").ap()
    stats_out = nc.dram_tensor("stats_out", [2, P, E], f32,
                               kind="ExternalOutput").ap()

    idx_view = idx_out.rearrange("(g b p) k -> g p b k", g=NGRP, b=GRP, p=P)
    wgt_view = wgt_out.rearrange("(g b p) k -> g p b k", g=NGRP, b=GRP, p=P)

    with tile.TileContext(nc) as tc:
        with ExitStack() as ctx:
            const = ctx.enter_context(tc.tile_pool(name="const", bufs=1))
            xin = ctx.enter_context(tc.tile_pool(name="xin", bufs=6))
            xtp = ctx.enter_context(tc.tile_pool(name="xtp", bufs=3))
            work = ctx.enter_context(tc.tile_pool(name="work", bufs=4))
            small = ctx.enter_context(tc.tile_pool(name="small", bufs=6))
            stage = ctx.enter_context(tc.tile_pool(name="stage", bufs=2))
            psum_t = ctx.enter_context(
                tc.tile_pool(name="psum_t", bufs=6, space="PSUM"))
            psum_l = ctx.enter_context(
                tc.tile_pool(name="psum_l", bufs=2, space="PSUM"))

            identity = const.tile([P, P], f32)
            make_identity(nc, identity)
            wt_s = const.tile([P, NCH, E], f32)
            nc.sync.dma_start(out=wt_s, in_=wt)
            acc_s = const.tile([P, E], f32)
            acc_h = const.tile([P, E], f32)
            nc.vector.memset(acc_s, 0.0)
            nc.vector.memset(acc_h, 0.0)

            for g in range(NGRP):
                idx_stage = stage.tile([P, GRP, TOPK], u32, tag="idx_stage")
                wgt_stage = stage.tile([P, GRP, TOPK], f32, tag="wgt_stage")
                for bi in range(GRP):
                    b = g * GRP + bi
                    xt = xin.tile([P, DIM], f32, tag="xin")
                    nc.sync.dma_start(out=xt, in_=x[b * P:(b + 1) * P, :])

                    # Transpose the block chunk-by-chunk: 4 chunks of
                    # [128, 128] per PSUM bank, then one copy per bank.
                    xT = xtp.tile([P, NCH, P], f32, tag="xtp")
                    for q in range(4):
                        pt = psum_t.tile([P, 4, P], f32, tag="pt")
                        for j in range(4):
                            c = q * 4 + j
                            nc.tensor.transpose(
                                pt[:, j, :], xt[:, c * P:(c + 1) * P], identity)
                        dst = xT[:, q * 4:(q + 1) * 4, :]
                        if q % 2 == 0:
                            nc.vector.tensor_copy(out=dst, in_=pt)
                        else:
                            nc.scalar.copy(out=dst, in_=pt)

                    lg = psum_l.tile([P, E], f32, tag="lg")
                    for c in range(NCH):
                        nc.tensor.matmul(lg, xT[:, c, :], wt_s[:, c, :],
                                         start=(c == 0), stop=(c == NCH - 1))

                    e_t = work.tile([P, E], f32, tag="e")
                    z = small.tile([P, 1], f32, tag="z")
                    nc.scalar.activation(e_t, lg,
                                         mybir.ActivationFunctionType.Exp,
                                         accum_out=z)
                    m8 = small.tile([P, TOPK], f32, tag="m8")
                    nc.vector.max(out=m8, in_=e_t)
                    nc.vector.max_index(out=idx_stage[:, bi, :], in_max=m8,
                                        in_values=e_t)
                    s8 = small.tile([P, 1], f32, tag="s8")
                    nc.vector.reduce_sum(s8, m8, axis=mybir.AxisListType.X)
                    r8 = small.tile([P, 1], f32, tag="r8")
                    nc.vector.reciprocal(r8, s8)
                    nc.vector.tensor_scalar_mul(wgt_stage[:, bi, :], m8, r8)
                    rz = small.tile([P, 1], f32, tag="rz")
                    nc.vector.reciprocal(rz, z)
                    # acc_s += e * (1/Z);  acc_h += (e >= 8th-largest e)
                    nc.vector.scalar_tensor_tensor(
                        out=acc_s, in0=e_t, scalar=rz, in1=acc_s,
                        op0=Alu.mult, op1=Alu.add)
                    nc.vector.scalar_tensor_tensor(
                        out=acc_h, in0=e_t, scalar=m8[:, TOPK - 1:TOPK],
                        in1=acc_h, op0=Alu.is_ge, op1=Alu.add)

                nc.sync.dma_start(out=idx_view[g], in_=idx_stage)
                nc.sync.dma_start(out=wgt_view[g], in_=wgt_stage)

            nc.sync.dma_start(out=stats_out[0], in_=acc_s)
            nc.sync.dma_start(out=stats_out[1], in_=acc_h)

    nc.compile()
    return nc


def _get_program():
    if "nc" not in _CACHE:
        _CACHE["nc"] = _build_program()
    return _CACHE["nc"]


def kernel(hidden_states, weight):
    from concourse.bass_utils import run_bass_kernel_spmd

    x = np.ascontiguousarray(np.asarray(hidden_states, dtype=np.float32))
    x = x.reshape(N_TOKENS, DIM)
    w = np.asarray(weight, dtype=np.float32)
    # W.T [DIM, E] -> [p, c, e] layout (dim = c*128 + p)
    wt = np.ascontiguousarray(w.T.reshape(NCH, P, E).transpose(1, 0, 2))

    nc = _get_program()
    in_maps = [
        {"x": x[c * TOK_PER_CORE:(c + 1) * TOK_PER_CORE], "wt": wt}
        for c in range(N_CORES)
    ]
    res = run_bass_kernel_spmd(nc, in_maps, core_ids=list(range(N_CORES)))
    outs = res.results

    topk_idx = np.concatenate(
        [o["idx_out"].view(np.int32) for o in outs], axis=0)
    topk_wgt = np.concatenate([o["wgt_out"] for o in outs], axis=0)

    # stats: [core, 2, 128, 64] -> per-core column sums -> per-batch sums
    colsums = np.stack([o["stats_out"] for o in outs]).sum(axis=2)  # [8, 2, 64]
    per_batch = colsums.reshape(BSZ, 2, 2, E).sum(axis=1)  # [4, 2, 64]
    s_sum = per_batch[:, 0]  # sum over seq of softmax scores  [4, 64]
    h_sum = per_batch[:, 1]  # top-8 histogram counts          [4, 64]
    ce = h_sum / np.float32(SEQ * TOPK / E)
    smean = s_sum / np.float32(SEQ)
    aux_loss = np.float32((ce * smean).sum(axis=1).mean() * ALPHA)

    return topk_idx, topk_wgt, aux_loss


# revision 7
# speedup vs baseline: 1.0107x; 1.0017x over previous
"""MoE gate (DeepSeek-style) Trainium2 Bass kernel.

Computation (per token): logits = x @ W.T over 64 experts; softmax; top-8
indices + renormalized top-8 weights; plus the seq-aux-loss statistics
(per-batch expert histogram of the top-8 picks and per-batch mean softmax
scores), reduced to a scalar aux loss.

Sharding: data parallel over tokens. hidden_states [4, 8192, 2048] is
flattened to [32768, 2048] and split into 8 contiguous shards of 4096
tokens (core c gets tokens of batch c//2, so per-batch stats reassemble
trivially on host). The [64, 2048] gate weight is replicated (host passes
it pre-transposed as [2048, 64]).

Per-core kernel (4096 tokens, 32 blocks of 128):
  - DMA x block [128 tok, 2048] to SBUF (1 MiB, contiguous).
  - PE-transposes the block 128x128 at a time into PSUM (fp32 has no DMA
    transpose on TRN2), copies to SBUF via DVE/ACT.
  - 16 accumulating fp32 matmuls: logits[128 tok, 64 exp] in PSUM
    (lhsT = x.T chunk, rhs = W.T chunk).
  - ACT: e = exp(logits) with accumulated row-sum Z (softmax denominator).
  - DVE: max8 -> top-8 values (descending, matches jax.lax.top_k order),
    max_index -> top-8 indices; renormalized weights = m8 / sum(m8)
    (the softmax denominator cancels); running stats via fused
    scalar_tensor_tensor: acc_s += e * (1/Z), acc_h += (e >= m8[:,7]).
  - Outputs staged in SBUF and stored every 8 blocks; the [128, 64] stat
    accumulators are stored once at the end (host sums the 128 partitions).

The host then concatenates the per-core topk outputs and finishes the
tiny aux-loss reduction ([8, 2, 64] -> scalar) in numpy.
"""

import numpy as np

P = 128
DIM = 2048
E = 64
TOPK = 8
BSZ, SEQ = 4, 8192
N_TOKENS = BSZ * SEQ
N_CORES = 8
TOK_PER_CORE = N_TOKENS // N_CORES  # 4096
NBLK = TOK_PER_CORE // P  # 32
NCH = DIM // P  # 16
GRP = 8  # blocks per output store
NGRP = NBLK // GRP
ALPHA = 0.1

_CACHE = {}


def _build_program():
    from contextlib import ExitStack

    import concourse.bacc as bacc
    import concourse.mybir as mybir
    import concourse.tile as tile
    from concourse.masks import make_identity

    f32 = mybir.dt.float32
    u32 = mybir.dt.uint32
    Alu = mybir.AluOpType

    nc = bacc.Bacc("TRN2", target_bir_lowering=False, debug=False,
                   num_devices=N_CORES)
    x = nc.dram_tensor("x", [TOK_PER_CORE, DIM], f32, kind="ExternalInput").ap()
    # host passes W.T pre-shuffled to [p, c, e] so this loads as one
    # contiguous DMA with 4 KiB per-partition descriptors
    wt = nc.dram_tensor("wt", [P, NCH, E], f32, kind="ExternalInput").ap()
    idx_out = nc.dram_tensor("idx_out", [TOK_PER_CORE, TOPK], u32,
                             kind="ExternalOutput").ap()
    wgt_out = nc.dram_tensor("wgt_out", [TOK_PER_CORE, TOPK], f32,
                             kind="ExternalOutput").ap()
    stats_out = nc.dram_tensor("stats_out", [2, P, E], f32,
                               kind="ExternalOutput").ap()

    idx_view = idx_out.rearrange("(g b p) k -> g p b k", g=NGRP, b=GRP, p=P)
    wgt_view = wgt_out.rearrange("(g b p) k -> g p b k", g=NGRP, b=GRP, p=P)

    with tile.TileContext(nc) as tc:
        with ExitStack() as ctx:
            const = ctx.enter_context(tc.tile_pool(name="const", bufs=1))
            xin = ctx.enter_context(tc.tile_pool(name="xin", bufs=6))
            xtp = ctx.enter_context(tc.tile_pool(name="xtp", bufs=3))
            work = ctx.enter_context(tc.tile_pool(name="work", bufs=4))
            small = ctx.enter_context(tc.tile_pool(name="small", bufs=6))
            stage = ctx.enter_context(tc.tile_pool(name="stage", bufs=2))
            psum_t = ctx.enter_context(
                tc.tile_pool(name="psum_t", bufs=6, space="PSUM"))
            psum_l = ctx.enter_context(
                tc.tile_pool(name="psum_l", bufs=2, space="PSUM"))

            identity = const.tile([P, P], f32)
            make_identity(nc, identity)
            wt_s = const.tile([P, NCH, E], f32)
            nc.sync.dma_start(out=wt_s, in_=wt)
            acc_s = const.tile([P, E], f32)
            acc_h = const.tile([P, E], f32)
            nc.vector.memset(acc_s, 0.0)
            nc.vector.memset(acc_h, 0.0)

            for g in range(NGRP):
                idx_stage = stage.tile([P, GRP, TOPK], u32, tag="idx_stage")
                wgt_stage = stage.tile([P, GRP, TOPK], f32, tag="wgt_stage")
                for bi in range(GRP):
                    b = g * GRP + bi
                    xt = xin.tile([P, DIM], f32, tag="xin")
                    if b < 2:
                        # split the pipeline-fill loads so the first
                        # transposes can start ~3us earlier
                        for q4 in range(4):
                            nc.sync.dma_start(
                                out=xt[:, q4 * 512:(q4 + 1) * 512],
                                in_=x[b * P:(b + 1) * P,
                                      q4 * 512:(q4 + 1) * 512])
                    else:
                        nc.sync.dma_start(out=xt, in_=x[b * P:(b + 1) * P, :])

                    # Transpose the block chunk-by-chunk: 4 chunks of
                    # [128, 128] per PSUM bank, then one copy per bank.
                    xT = xtp.tile([P, NCH, P], f32, tag="xtp")
                    for q in range(4):
                        pt = psum_t.tile([P, 4, P], f32, tag="pt")
                        for j in range(4):
                            c = q * 4 + j
                            nc.tensor.transpose(
                                pt[:, j, :], xt[:, c * P:(c + 1) * P], identity)
                        dst = xT[:, q * 4:(q + 1) * 4, :]
                        if q % 2 == 0:
                            nc.vector.tensor_copy(out=dst, in_=pt)
                        else:
                            nc.scalar.copy(out=dst, in_=pt)

                    lg = psum_l.tile([P, E], f32, tag="lg")
                    for c in range(NCH):
                        nc.tensor.matmul(lg, xT[:, c, :], wt_s[:, c, :],
                                         start=(c == 0), stop=(c == NCH - 1))

                    e_t = work.tile([P, E], f32, tag="e")
                    z = small.tile([P, 1], f32, tag="z")
                    nc.scalar.activation(e_t, lg,
                                         mybir.ActivationFunctionType.Exp,
                                         accum_out=z)
                    m8 = small.tile([P, TOPK], f32, tag="m8")
                    nc.vector.max(out=m8, in_=e_t)
                    nc.vector.max_index(out=idx_stage[:, bi, :], in_max=m8,
                                        in_values=e_t)
                    s8 = small.tile([P, 1], f32, tag="s8")
                    nc.vector.reduce_sum(s8, m8, axis=mybir.AxisListType.X)
                    r8 = small.tile([P, 1], f32, tag="r8")
                    nc.vector.reciprocal(r8, s8)
                    nc.vector.tensor_scalar_mul(wgt_stage[:, bi, :], m8, r8)
                    rz = small.tile([P, 1], f32, tag="rz")
                    nc.vector.reciprocal(rz, z)
                    # acc_s += e * (1/Z);  acc_h += (e >= 8th-largest e)
                    nc.vector.scalar_tensor_tensor(
                        out=acc_s, in0=e_t, scalar=rz, in1=acc_s,
                        op0=Alu.mult, op1=Alu.add)
                    nc.vector.scalar_tensor_tensor(
                        out=acc_h, in0=e_t, scalar=m8[:, TOPK - 1:TOPK],
                        in1=acc_h, op0=Alu.is_ge, op1=Alu.add)

                nc.sync.dma_start(out=idx_view[g], in_=idx_stage)
                nc.sync.dma_start(out=wgt_view[g], in_=wgt_stage)

            nc.sync.dma_start(out=stats_out[0], in_=acc_s)
            nc.sync.dma_start(out=stats_out[1], in_=acc_h)

    nc.compile()
    return nc


def _get_program():
    if "nc" not in _CACHE:
        _CACHE["nc"] = _build_program()
    return _CACHE["nc"]


def kernel(hidden_states, weight):
    from concourse.bass_utils import run_bass_kernel_spmd

    x = np.ascontiguousarray(np.asarray(hidden_states, dtype=np.float32))
    x = x.reshape(N_TOKENS, DIM)
    w = np.asarray(weight, dtype=np.float32)
    # W.T [DIM, E] -> [p, c, e] layout (dim = c*128 + p)
    wt = np.ascontiguousarray(w.T.reshape(NCH, P, E).transpose(1, 0, 2))

    nc = _get_program()
    in_maps = [
        {"x": x[c * TOK_PER_CORE:(c + 1) * TOK_PER_CORE], "wt": wt}
        for c in range(N_CORES)
    ]
    res = run_bass_kernel_spmd(nc, in_maps, core_ids=list(range(N_CORES)))
    outs = res.results

    topk_idx = np.concatenate(
        [o["idx_out"].view(np.int32) for o in outs], axis=0)
    topk_wgt = np.concatenate([o["wgt_out"] for o in outs], axis=0)

    # stats: [core, 2, 128, 64] -> per-core column sums -> per-batch sums
    colsums = np.stack([o["stats_out"] for o in outs]).sum(axis=2)  # [8, 2, 64]
    per_batch = colsums.reshape(BSZ, 2, 2, E).sum(axis=1)  # [4, 2, 64]
    s_sum = per_batch[:, 0]  # sum over seq of softmax scores  [4, 64]
    h_sum = per_batch[:, 1]  # top-8 histogram counts          [4, 64]
    ce = h_sum / np.float32(SEQ * TOPK / E)
    smean = s_sum / np.float32(SEQ)
    aux_loss = np.float32((ce * smean).sum(axis=1).mean() * ALPHA)

    return topk_idx, topk_wgt, aux_loss
